# revision 15
# baseline (speedup 1.0000x reference)
"""Trainium2 Bass kernel for nn_CGCNNModel (CGCNN-style GCN + sum-pool + MLP heads).

Math: the 3 GCN layers and the sum-pooling are all linear, so per graph

    pooled = u3^T x W0 W1 W2 + sum(u2) * (b0 W1 W2) + sum(u1) * (b1 W2) + N * b2

with u1 = An 1, u2 = An u1, u3 = An u2, An = D (C+I) D, D = diag(1/sqrt(deg+1)),
x = [node_feat | bond_feat].  This is an exact reassociation (verified to ~1e-6
against the reference).  The device reads every input byte (conn, bond, nf) and
does all data-dependent work: degree reduction, the 3 An-matvecs, the x^T u3
sweeps, pooling scalars, softplus and the head MLP.  Host only preprocesses
weights (W0@W1@W2 etc.) and shards the batch over 8 cores (16 graphs each).

Layout: node index i <-> (p, c) with i = 4p + c; all per-node vectors live as
[128, 4] SBUF tiles ("col4"), matrices as [128, 4, 512] (row 4p+c on partition
p, chunk c).  An-matvecs run on PE as M=1 matmuls (v col as stationary, C as
moving, N=512, fp32r) accumulating a [1, 512] PSUM row; the row is copied to
SBUF and reshaped back to col4 by a tiny SBUF->SBUF DMA.
"""

import os

import numpy as np

S, N, P = 128, 512, 92
NCORES = 8
GPC = S // NCORES  # graphs per core
NCHUNK = 4  # 512 = 4 * 128
HID = 256
NHEAD = 103  # 100 + 1 + 1 + 1

LAST_PROFILE = {}


def _build_program():
    import concourse.mybir as mybir
    import concourse.tile as tile
    from concourse import bacc
    from contextlib import ExitStack

    dt = mybir.dt
    f32 = dt.float32
    f16 = dt.float16
    AF = mybir.ActivationFunctionType

    nc = bacc.Bacc(None, target_bir_lowering=False, debug=False)

    conn_e = nc.declare_dram_parameter("conn", [GPC, N, N], dt.int32, isOutput=False)
    bond_e = nc.declare_dram_parameter("bond", [GPC, N, N], f32, isOutput=False)
    nf_e = nc.declare_dram_parameter("nf", [GPC, N, P], f32, isOutput=False)
    waug_e = nc.declare_dram_parameter("Waug", [128, 5, 128], f16, isOutput=False)
    wh_e = nc.declare_dram_parameter("Wh", [128, HID], f16, isOutput=False)
    bh2_e = nc.declare_dram_parameter("bh2", [128, 2], f32, isOutput=False)
    whd_e = nc.declare_dram_parameter("Whead", [128, 2, NHEAD], f16, isOutput=False)
    bhd_e = nc.declare_dram_parameter("bhead", [NHEAD, 1], f32, isOutput=False)
    out_e = nc.declare_dram_parameter("out", [NHEAD, GPC], f32, isOutput=True)

    with tile.TileContext(nc) as tc, ExitStack() as ctx:
        const = ctx.enter_context(tc.tile_pool(name="const", bufs=1))
        io = ctx.enter_context(tc.tile_pool(name="io", bufs=4))
        vec = ctx.enter_context(tc.tile_pool(name="vec", bufs=4))
        rows = ctx.enter_context(tc.tile_pool(name="rows", bufs=4))
        ps = ctx.enter_context(tc.tile_pool(name="ps", bufs=1, space="PSUM"))

        # ---- constants / persistent tiles ----
        waug = const.tile([128, 5, 128], f16, name="waug")
        nc.sync.dma_start(out=waug, in_=waug_e[:])
        wh = const.tile([128, HID], f16, name="wh")
        nc.sync.dma_start(out=wh, in_=wh_e[:])
        bh2 = const.tile([128, 2], f32, name="bh2")
        nc.sync.dma_start(out=bh2, in_=bh2_e[:])
        whd = const.tile([128, 2, NHEAD], f16, name="whd")
        nc.sync.dma_start(out=whd, in_=whd_e[:])
        bhd = const.tile([128, 1], f32, name="bhd")
        nc.sync.dma_start(out=bhd[0:NHEAD, :], in_=bhd_e[:])
        ones_c = const.tile([128, 1], f16, name="ones_c")
        nc.vector.memset(ones_c, 1.0)
        # w_aug: per-graph augmented moment vectors (columns), 5 chunks of 128.
        # chunks 0..3: bond moments (k = 4p + cc); chunk 4: nf moments (p<92),
        # sigma2 (92), sigma1 (93), const 1.0 (94), zeros (95..127).
        w_aug = const.tile([128, 5, GPC], f16, name="w_aug")

        # ---- software-pipelined graph loop ----
        # stage offsets: S0 conn load + deg/dinv; S1..S3 matvecs; S2 also
        # issues bond/nf loads; S4 w-sweeps.  Emission interleaves stages
        # across graphs so per-engine FIFO order has no serial-chain stalls.
        st = [dict() for _ in range(GPC)]

        def s0(g):
            C = io.tile([128, NCHUNK, N], f16, tag="C", bufs=5, name=f"C{g}")
            # gpsimd (SWDGE) DMA converts int32 -> f32 in flight
            nc.gpsimd.dma_start(out=C, in_=conn_e[g].rearrange("(p c) j -> p c j", c=NCHUNK))
            deg = vec.tile([128, NCHUNK], f32, tag="deg", bufs=3, name=f"deg{g}")
            # row sums on ACT (copy with free-axis accumulate) to keep DVE light
            scr = rows.tile([128, N], f32, tag="scr", bufs=2, name=f"scr{g}")
            for c in range(NCHUNK):
                nc.scalar.activation(out=scr, in_=C[:, c, :], func=AF.Copy, accum_out=deg[:, c : c + 1])
            sq = vec.tile([128, NCHUNK], f32, tag="sq", bufs=3, name=f"sq{g}")
            # sqrt(deg + 1): +1 is the self loop
            nc.scalar.activation(out=sq, in_=deg, func=AF.Sqrt, bias=1.0, scale=1.0)
            dinv32 = vec.tile([128, NCHUNK], f32, tag="dinv32", bufs=3, name=f"dinv32{g}")
            nc.vector.reciprocal(out=dinv32, in_=sq)
            dinv = vec.tile([128, NCHUNK], f16, tag="dinv", bufs=5, name=f"dinv{g}")
            nc.vector.tensor_copy(dinv, dinv32)
            st[g].update(C=C, dinv=dinv, v=dinv)

        def s_mv(g, k):
            C, dinv, v = st[g]["C"], st[g]["dinv"], st[g]["v"]
            r_ps = ps.tile([1, N], f32, tag="r_ps", bufs=3, name=f"rps{g}_{k}")
            for c in range(NCHUNK):
                nc.tensor.matmul(
                    r_ps,
                    lhsT=v[:, c : c + 1],
                    rhs=C[:, c, :],
                    start=(c == 0),
                    stop=(c == NCHUNK - 1),
                )
            r_row = rows.tile([1, N], f16, tag="r_row", bufs=3, name=f"rrow{g}_{k}")
            nc.vector.tensor_copy(r_row, r_ps)
            r_col = vec.tile([128, NCHUNK], f16, tag="r_col", bufs=3, name=f"rcol{g}_{k}")
            nc.scalar.dma_start(out=r_col, in_=r_row)  # ACT HWDGE ring
            t = vec.tile([128, NCHUNK], f16, tag="t", bufs=3, name=f"t{g}_{k}")
            nc.vector.tensor_add(t, r_col, v)
            u = vec.tile([128, NCHUNK], f16, tag=f"u{k}", bufs=5, name=f"u{g}_{k}")
            nc.vector.tensor_mul(u, t, dinv)
            if k < 2:
                # per-partition sum of u for sigma_k (finished by one PE dot)
                uacc = vec.tile([128, 1], f32, tag=f"uacc{k}", bufs=5, name=f"uacc{g}_{k}")
                nc.vector.tensor_reduce(out=uacc, in_=u, axis=mybir.AxisListType.X, op=mybir.AluOpType.add)
                uacc16 = vec.tile([128, 1], f16, tag=f"uacc16_{k}", bufs=5, name=f"uacc16_{g}_{k}")
                nc.vector.tensor_copy(uacc16, uacc)
                st[g][f"uacc{k}"] = uacc16
                v2 = vec.tile([128, NCHUNK], f16, tag=f"v{k}", bufs=3, name=f"v{g}_{k}")
                nc.vector.tensor_mul(v2, u, dinv)
                st[g]["v"] = v2
            st[g][f"u{k}"] = u

        def s_load_x(g):
            bond = io.tile([128, NCHUNK, N], f16, tag="bond", bufs=3, name=f"bond{g}")
            nc.gpsimd.dma_start(out=bond, in_=bond_e[g].rearrange("(p c) j -> p c j", c=NCHUNK))
            nft = io.tile([128, NCHUNK, P], f16, tag="nf", bufs=3, name=f"nf{g}")
            nc.gpsimd.dma_start(out=nft, in_=nf_e[g].rearrange("(p c) f -> p c f", c=NCHUNK))
            st[g].update(bond=bond, nft=nft)

        def s4(g):
            u1, u2, u3 = st[g]["u0"], st[g]["u1"], st[g]["u2"]
            bond, nft = st[g]["bond"], st[g]["nft"]
            wb_ps = ps.tile([1, N], f32, tag="wb_ps", bufs=2, name=f"wbps{g}")
            for c in range(NCHUNK):
                nc.tensor.matmul(
                    wb_ps,
                    lhsT=u3[:, c : c + 1],
                    rhs=bond[:, c, :],
                    start=(c == 0),
                    stop=(c == NCHUNK - 1),
                )
            wn_ps = ps.tile([1, 96], f32, tag="wn_ps", bufs=1, name=f"wnps{g}")
            for c in range(NCHUNK):
                nc.tensor.matmul(
                    wn_ps[:, 0:P],
                    lhsT=u3[:, c : c + 1],
                    rhs=nft[:, c, :],
                    start=(c == 0),
                    stop=(c == NCHUNK - 1),
                )
            # sigma2 = sum(u2), sigma1 = sum(u1): finish the fused per-partition
            # sums with one PE dot each
            nc.tensor.matmul(
                wn_ps[:, 92:93],
                lhsT=st[g]["uacc1"],
                rhs=ones_c,
                start=True,
                stop=True,
            )
            nc.tensor.matmul(
                wn_ps[:, 93:94],
                lhsT=st[g]["uacc0"],
                rhs=ones_c,
                start=True,
                stop=True,
            )

            wb_row = rows.tile([1, N], f16, tag="wb_row", bufs=2, name=f"wbrow{g}")
            nc.vector.tensor_copy(wb_row, wb_ps)
            nc.scalar.dma_start(out=w_aug[:, 0:4, g : g + 1], in_=wb_row)
            # full chunk-4 column: nf moments, sigma2, sigma1, const 1.0, zeros
            wn_row = rows.tile([1, 128], f16, tag="wn_row", bufs=2, name=f"wnrow{g}")
            nc.vector.tensor_copy(wn_row[:, 0:94], wn_ps[:, 0:94])
            nc.vector.memset(wn_row[:, 94:95], 1.0)
            nc.vector.memset(wn_row[:, 95:128], 0.0)
            nc.scalar.dma_start(out=w_aug[:, 4:5, g : g + 1], in_=wn_row)
            st[g].clear()

        for step in range(GPC + 4):
            for off, fn in ((4, s4), (3, lambda g: s_mv(g, 2)), (2, lambda g: (s_mv(g, 1), s_load_x(g))), (1, lambda g: s_mv(g, 0)), (0, s0)):
                g = step - off
                if 0 <= g < GPC:
                    fn(g)

        # ---- final: pooled^T = Waug^T w_aug ; softplus ; MLP heads ----
        pooled_ps = ps.tile([128, GPC], f32, tag="pooled", name="pooled_ps")
        for cc in range(5):
            nc.tensor.matmul(
                pooled_ps,
                lhsT=waug[:, cc, :],
                rhs=w_aug[:, cc, :],
                start=(cc == 0),
                stop=(cc == 4),
            )
        # softplus(x) = relu(x) + ln(1 + exp(-|x|))  (no Softplus HW table)
        ax = rows.tile([128, GPC], f32, tag="ax", name="ax")
        nc.scalar.activation(out=ax, in_=pooled_ps, func=AF.Abs)
        ex = rows.tile([128, GPC], f32, tag="ex", name="ex")
        nc.scalar.activation(out=ex, in_=ax, func=AF.Exp, scale=-1.0)
        ln1p = rows.tile([128, GPC], f32, tag="ln1p", name="ln1p")
        nc.scalar.activation(out=ln1p, in_=ex, func=AF.Ln, bias=1.0, scale=1.0)
        rl = rows.tile([128, GPC], f32, tag="rl", name="rl")
        nc.scalar.activation(out=rl, in_=pooled_ps, func=AF.Relu)
        sp = rows.tile([128, GPC], f16, tag="sp", name="sp")
        nc.vector.tensor_add(sp, ln1p, rl)

        hT_ps = ps.tile([128, 2 * GPC], f32, tag="hT", name="hT_ps")
        for mc in range(2):
            nc.tensor.matmul(
                hT_ps[:, mc * GPC : (mc + 1) * GPC],
                lhsT=wh[:, mc * 128 : (mc + 1) * 128],
                rhs=sp,
                start=True,
                stop=True,
            )
        hTs = rows.tile([128, 2, GPC], f16, tag="hTs", name="hTs")
        for mc in range(2):
            nc.scalar.activation(
                out=hTs[:, mc, :],
                in_=hT_ps[:, mc * GPC : (mc + 1) * GPC],
                func=AF.Identity,
                bias=bh2[:, mc : mc + 1],
                scale=1.0,
            )

        heads_ps = ps.tile([128, GPC], f32, tag="pooled", name="heads_ps")
        for mc in range(2):
            nc.tensor.matmul(
                heads_ps[0:NHEAD, :],
                lhsT=whd[:, mc, :],
                rhs=hTs[:, mc, :],
                start=(mc == 0),
                stop=(mc == 1),
            )
        outs_sb = rows.tile([128, GPC], f32, tag="outs", name="outs_sb")
        nc.scalar.activation(
            out=outs_sb[0:NHEAD, :],
            in_=heads_ps[0:NHEAD, :],
            func=AF.Identity,
            bias=bhd[0:NHEAD, :],
            scale=1.0,
        )
        nc.sync.dma_start(out=out_e[:], in_=outs_sb[0:NHEAD, :])

    nc.compile()
    return nc


def _softplus(x):
    return np.logaddexp(0.0, x)


def _reference_numpy(node_feat, bond_feat, connectivity, batchAssign,
                     W0, b0, W1, b1, W2, b2, Wh, bh, Wi, bi, We, be, Wlb, blb, Wub, bub):
    """Exact fallback replicating reference.py for inputs that break the
    fast-path assumptions (non-symmetric connectivity / non-uniform segments)."""
    s, n, _ = node_feat.shape
    x = np.concatenate([node_feat, bond_feat[:, :n, :n]], axis=-1).astype(np.float32)
    Ahat = (connectivity != 0).astype(np.float32) + np.eye(n, dtype=np.float32)[None]
    deg = Ahat.sum(axis=1)
    dinv = 1.0 / np.sqrt(deg)
    An = Ahat * dinv[:, :, None] * dinv[:, None, :]
    AnT = An.transpose(0, 2, 1)
    for W, b in ((W0, b0), (W1, b1), (W2, b2)):
        x = np.matmul(AnT, x @ W) + b
    flat = x.reshape(s * n, -1)
    pooled = np.zeros((s, flat.shape[1]), np.float32)
    idx = np.clip(batchAssign, 0, s - 1)
    valid = (batchAssign >= 0) & (batchAssign < s)
    np.add.at(pooled, idx[valid], flat[valid])
    h = _softplus(pooled).astype(np.float32) @ Wh + bh
    return (h @ Wi + bi, h @ We + be, h @ Wlb + blb, h @ Wub + bub)


def _prepare(inputs):
    """Validate fast-path assumptions, build the Bass program and per-core
    input maps.  Returns (nc, in_maps) or None if the fallback must run."""
    node_feat = np.ascontiguousarray(np.asarray(inputs["node_feat"], dtype=np.float32))
    bond_feat = np.ascontiguousarray(np.asarray(inputs["bond_feat"], dtype=np.float32))
    conn = np.ascontiguousarray(np.asarray(inputs["connectivity"], dtype=np.int32))
    ba = np.asarray(inputs["batchAssign"])
    W0, b0 = np.asarray(inputs["W0"]), np.asarray(inputs["b0"])
    W1, b1 = np.asarray(inputs["W1"]), np.asarray(inputs["b1"])
    W2, b2 = np.asarray(inputs["W2"]), np.asarray(inputs["b2"])
    Wh, bh = np.asarray(inputs["Wh"]), np.asarray(inputs["bh"])
    Wi, bi = np.asarray(inputs["Wi"]), np.asarray(inputs["bi"])
    We, be = np.asarray(inputs["We"]), np.asarray(inputs["be"])
    Wlb, blb = np.asarray(inputs["Wlb"]), np.asarray(inputs["blb"])
    Wub, bub = np.asarray(inputs["Wub"]), np.asarray(inputs["bub"])

    fast = (
        node_feat.shape == (S, N, P)
        and bond_feat.shape == (S, N, N)
        and conn.shape == (S, N, N)
        and np.array_equal(ba, np.repeat(np.arange(S, dtype=ba.dtype), N))
        and conn.min() >= 0
        and conn.max() <= 1
        and np.array_equal(conn, conn.transpose(0, 2, 1))
    )
    if not fast:
        return None

    # ---- host-side weight preprocessing (weights only, no data) ----
    W012 = (W0.astype(np.float64) @ W1.astype(np.float64) @ W2.astype(np.float64))
    c1 = b0.astype(np.float64) @ W1.astype(np.float64) @ W2.astype(np.float64)
    c2 = b1.astype(np.float64) @ W2.astype(np.float64)
    W012 = W012.astype(np.float32)
    Waug = np.zeros((128, 5, 128), np.float16)
    Waug[:, 0:4, :] = W012[P:].reshape(128, 4, 128)  # bond part, k = 4p + cc
    Waug[0:P, 4, :] = W012[:P]  # nf part
    Waug[P, 4, :] = c1.astype(np.float32)  # pairs sigma2
    Waug[P + 1, 4, :] = c2.astype(np.float32)  # pairs sigma1
    Waug[P + 2, 4, :] = (N * b2).astype(np.float32)  # pairs const 1.0
    bh2 = np.ascontiguousarray(bh.reshape(2, 128).T.astype(np.float32))
    Whead_full = np.hstack([Wi, We, Wlb, Wub]).astype(np.float32)  # (256, 103)
    Whead = np.ascontiguousarray(Whead_full.reshape(2, 128, NHEAD).transpose(1, 0, 2).astype(np.float16))
    bhead = np.concatenate([bi, be, blb, bub]).astype(np.float32).reshape(NHEAD, 1)
    Wh_ = np.ascontiguousarray(Wh.astype(np.float16))

    nc = _build_program()

    in_maps = []
    for c in range(NCORES):
        sl = slice(c * GPC, (c + 1) * GPC)
        in_maps.append(
            dict(
                conn=conn[sl],
                bond=bond_feat[sl],
                nf=node_feat[sl],
                Waug=Waug,
                Wh=Wh_,
                bh2=bh2,
                Whead=Whead,
                bhead=bhead,
            )
        )
    return nc, in_maps


def kernel(**inputs):
    prep = _prepare(inputs)
    if prep is None:
        names = ["node_feat", "bond_feat", "connectivity", "batchAssign",
                 "W0", "b0", "W1", "b1", "W2", "b2", "Wh", "bh",
                 "Wi", "bi", "We", "be", "Wlb", "blb", "Wub", "bub"]
        return _reference_numpy(*[np.asarray(inputs[n]) for n in names])
    nc, in_maps = prep

    from concourse.bass_utils import run_bass_kernel_spmd

    trace = os.environ.get("KERNEL_TRACE", "") == "1"
    res = run_bass_kernel_spmd(nc, in_maps, core_ids=list(range(NCORES)), trace=trace)
    if trace:
        LAST_PROFILE.clear()
        LAST_PROFILE.update(
            exec_time_ns=res.exec_time_ns,
            mean_exec_time_ns=res.mean_exec_time_ns,
            trace=(res.instructions_and_trace[1] if res.instructions_and_trace else None),
        )

    outT = np.concatenate([res.results[c]["out"] for c in range(NCORES)], axis=1)
    out = np.ascontiguousarray(outT.T.astype(np.float32))  # (128, 103)
    return (
        out[:, :100].copy(),
        out[:, 100:101].copy(),
        out[:, 101:102].copy(),
        out[:, 102:103].copy(),
    )


if __name__ == "__main__":
    import jax

    jax.config.update("jax_platforms", "cpu")
    import reference as ref

    ins = {k: np.asarray(v) for k, v in ref.setup_inputs().items()}
    outs = kernel(**ins)
    print([o.shape for o in outs])
